# revision 11
# baseline (speedup 1.0000x reference)
"""Trainium2 Bass kernel for PhaseCoherenceComputer.

coherence[b,h,q,k] = mean_d cos(phases_q[b,h,q,d] - phases_k[b,h,k,d])
                   = (cos_q @ cos_k^T + sin_q @ sin_k^T) / 64

Shapes: phases_q/k [2, 8, 2048, 64] f32 -> out [2, 8, 2048, 2048] f32.

Strategy (8 NeuronCores, data-parallel over the 16 (b,h) pairs, 2 per core):
- f16 everywhere off-chip (harness tolerance is 2e-2, f16 adds ~2e-4):
  per core 16.8 MB out + 1.5 MB in vs 33.5 MB + 2 MB for the f32
  baseline. The kernel is HBM-write-bound at ~358 GB/s/core, so bytes =
  time; everything else is pipelined under the write stream.
- Pair 0 (ramp-critical): host ships range-reduced phases r in [-pi,pi]
  as f16 [64, S] half-tensor chunks; a DVE sign-bit clear builds |r| in
  partitions 0:64 (r in 64:128) and one Sin activation with
  per-partition (scale, bias) = (-1, pi/2)/(+1, 0) yields
  U = [cos^T; sin^T] (cos r = sin(pi/2 - |r|), arguments inside the
  accurate [-pi/2, pi/2] spline range).
- Pair 1: host pre-packs [pi/2-|r|; r] as full [128, S] blocks (plain
  Sin, no abs) so its input DMAs use all 16 SDMA ports. Its raw tiles
  alias pair-0's (raw pool bufs=1): the WAR dependency delays these
  DMAs until pair-0's sins finish, so they never steal wire time from
  the ramp-critical pair-0 input, and instead ride the saturated output
  stream. Pair-1 sins are spread one half-sin per pair-0 block.
- One K=128 f16 matmul per [128 q x 512 k] output block. PSUM is carved
  into four [128, 1024] half-tiles (2 banks each): per q-tile, psA
  holds k-blocks 0-1 and psB k-blocks 2-3, so VectorE (psA) and ACT
  (psB) recycle PSUM independently; the PSUM chain (matmul + one
  half-evac ~2.4 us per 2 tiles) stays under the DMA drain period.
  Evacuation applies the 1/64 scale and converts to f16.
- Output DMA: 2 q-tiles per [128, 2*S] f16 SBUF block, one 1 MB
  sync-ring (HWDGE) DMA with 8 KB contiguous per-partition descriptors
  (DRAM layout [8 blocks, 128, 2*S] per pair; host unpermutes). All
  output DMAs ride the otherwise-idle SP ring so ACT compute never
  delays an issue. The first and last blocks stream as 4 x 256 KB
  quarter-DMAs fired per half-evac, starting the HBM write stream ~2 us
  earlier and shrinking the final drain.
"""

import sys

import numpy as np

try:
    import concourse.bacc as bacc
except ImportError:  # fresh interpreter without the axon site path
    for _p in ("/opt/trn_rl_repo", "/root/.axon_site/_ro/trn_rl_repo"):
        if _p not in sys.path:
            sys.path.insert(0, _p)
    import concourse.bacc as bacc

import concourse.mybir as mybir
import concourse.tile as tile
from concourse.bass_utils import run_bass_kernel_spmd

F32 = mybir.dt.float32
F16 = mybir.dt.float16
U16 = mybir.dt.uint16
UV_DT = F16  # matmul operand dtype
OUT_DT = F16  # device-side output dtype (host upcasts to f32)

B, H, S, D = 2, 8, 2048, 64
N_CORES = 8
PAIRS_PER_CORE = (B * H) // N_CORES  # 2
Q_TILE = 128  # output rows per matmul (PSUM partitions)
K_TILE = 512  # output cols per matmul
N_QT = S // Q_TILE  # 16
BLK = 2  # q-tiles per output DMA block (1 MB f16)
N_BLK = N_QT // BLK  # 8
HC = S // 2  # half-row chunk for input DMA / sin
EC = 2 * K_TILE  # evac chunk (one PSUM half-tile)

_NC_CACHE = {}


def build_kernel():
    """Per-core SPMD program. pin0 [2, 64, S] f16: pair-0 range-reduced
    transposed phases (tensor 0 = k-phases/v, 1 = q-phases/u). pin1
    [2, 128, S] f16: pair-1 packed [pi/2-|r|; r] blocks. Output out
    [PAIRS, N_BLK, 128, BLK*S] f16: block j holds q-tiles
    BLK*j..BLK*j+BLK-1 side by side."""
    nc = bacc.Bacc("TRN2", target_bir_lowering=False, debug=False)
    pin0 = nc.dram_tensor("pin0", [2, 64, S], F16, kind="ExternalInput")
    pin1 = nc.dram_tensor("pin1", [2, 128, S], F16, kind="ExternalInput")
    out = nc.dram_tensor(
        "out", [PAIRS_PER_CORE, N_BLK, 128, BLK * S], OUT_DT, kind="ExternalOutput"
    )
    SIN = mybir.ActivationFunctionType.Sin

    with tile.TileContext(nc) as tc:
        with (
            tc.tile_pool(name="const", bufs=1) as cpool,
            tc.tile_pool(name="raw", bufs=1) as rawpool,
            tc.tile_pool(name="uv", bufs=2) as uvpool,
            tc.tile_pool(name="ot", bufs=3) as opool,
            tc.tile_pool(name="psum", bufs=2, space="PSUM") as ppool,
        ):
            # Per-partition Sin affine for pair 0: top half cos via
            # sin(pi/2 - |r|), bottom half sin via sin(r).
            bias = cpool.tile([128, 1], F32)
            scale = cpool.tile([128, 1], F32)
            nc.vector.memset(bias[0:64, :], np.pi / 2)
            nc.vector.memset(bias[64:128, :], 0.0)
            nc.vector.memset(scale[0:64, :], -1.0)
            nc.vector.memset(scale[64:128, :], 1.0)

            raws = {}
            uvs = {}
            for p in range(PAIRS_PER_CORE):
                raws[p] = (
                    rawpool.tile([128, S], F16, tag="vraw", name="vraw"),
                    rawpool.tile([128, S], F16, tag="uraw", name="uraw"),
                )
                uvs[p] = (
                    uvpool.tile([128, S], UV_DT, tag="v", name="v"),
                    uvpool.tile([128, S], UV_DT, tag="u", name="u"),
                )

            # Pair-0 inputs in half-tensor chunks across both HWDGE rings
            # so the first sins start as early as possible. The first
            # q-tile's psA matmuls need v h0 + u cols 0:128, psB needs v h1.
            nc.sync.dma_start(out=raws[0][0][64:128, 0:HC], in_=pin0[0, :, 0:HC])
            nc.scalar.dma_start(out=raws[0][1][64:128, 0:HC], in_=pin0[1, :, 0:HC])
            nc.sync.dma_start(out=raws[0][0][64:128, HC:S], in_=pin0[0, :, HC:S])
            nc.scalar.dma_start(out=raws[0][1][64:128, HC:S], in_=pin0[1, :, HC:S])
            # Pair-1 inputs on the sync ring: the WAR on the aliased raw
            # buffers (bufs=1) holds these until pair-0's sins are done, so
            # they ride along the output stream, full-width.
            nc.sync.dma_start(out=raws[1][0][:], in_=pin1[0])
            nc.sync.dma_start(out=raws[1][1][:], in_=pin1[1])

            def abs_step(t, h):
                hs = slice(h * HC, (h + 1) * HC)
                nc.vector.tensor_scalar(
                    raws[0][t][0:64, hs].bitcast(U16),
                    raws[0][t][64:128, hs].bitcast(U16),
                    0x7FFF,
                    None,
                    mybir.AluOpType.bitwise_and,
                )

            def sin_step(p, t, h):
                hs = slice(h * HC, (h + 1) * HC)
                if p == 0:
                    nc.scalar.activation(
                        uvs[0][t][:, hs], raws[0][t][:, hs], SIN,
                        bias=bias[:], scale=scale[:],
                    )
                else:  # host-packed: plain sin
                    nc.scalar.activation(uvs[1][t][:, hs], raws[1][t][:, hs], SIN)

            for t, h in ((0, 0), (1, 0), (0, 1), (1, 1)):
                abs_step(t, h)
                sin_step(0, t, h)

            def q_tile(p, q, ot, col0, dma_quarters):
                v, u = uvs[p][0], uvs[p][1]
                us = u[:, q * Q_TILE : (q + 1) * Q_TILE]
                psA = ppool.tile([128, EC], F32, tag="psA", name="psA")
                psB = ppool.tile([128, EC], F32, tag="psB", name="psB")
                for k in range(2):
                    nc.tensor.matmul(
                        psA[:, k * K_TILE : (k + 1) * K_TILE],
                        us,
                        v[:, k * K_TILE : (k + 1) * K_TILE],
                        start=True,
                        stop=True,
                    )
                for k in range(2):
                    nc.tensor.matmul(
                        psB[:, k * K_TILE : (k + 1) * K_TILE],
                        us,
                        v[:, (k + 2) * K_TILE : (k + 3) * K_TILE],
                        start=True,
                        stop=True,
                    )
                nc.vector.tensor_scalar_mul(ot[:, col0 : col0 + EC], psA[:], 1.0 / D)
                if dma_quarters is not None:
                    nc.sync.dma_start(
                        out=dma_quarters[col0 : col0 + EC], in_=ot[:, col0 : col0 + EC]
                    )
                nc.scalar.mul(ot[:, col0 + EC : col0 + 2 * EC], psB[:], 1.0 / D)
                if dma_quarters is not None:
                    nc.sync.dma_start(
                        out=dma_quarters[col0 + EC : col0 + 2 * EC],
                        in_=ot[:, col0 + EC : col0 + 2 * EC],
                    )

            # Pair-1 half-sins spread one per block through pair-0's
            # q-loop, after its (WAR-delayed) input DMAs have landed and
            # early enough to finish before pair-0's last block.
            prep1 = {8: (1, 0, 0), 10: (1, 0, 1), 12: (1, 1, 0), 14: (1, 1, 1)}

            for p in range(PAIRS_PER_CORE):
                for blk in range(N_BLK):
                    ot = opool.tile([128, BLK * S], OUT_DT, tag="ot", name="ot")
                    split = (p == 0 and blk == 0) or (
                        p == PAIRS_PER_CORE - 1 and blk == N_BLK - 1
                    )
                    dq = None
                    if split:
                        class _Q:  # column-sliced DMA target for this block
                            def __getitem__(_s, cols):
                                return out[p, blk, :, cols]
                        dq = _Q()
                    for j in range(BLK):
                        q = blk * BLK + j
                        q_tile(p, q, ot, j * S, dq)
                        if p == 0 and q in prep1:
                            sin_step(*prep1[q])
                    if not split:
                        nc.sync.dma_start(out=out[p, blk], in_=ot[:])
    nc.compile()
    return nc


def _prep(ph):
    """[16, S, D] phases -> [16, 64, S] f64 range-reduced transposed."""
    pht = ph.astype(np.float64).transpose(0, 2, 1)  # [16, D, S]
    return np.mod(pht + np.pi, 2 * np.pi) - np.pi


def kernel(phases_q, phases_k, _trace=False):
    pq = np.asarray(phases_q, dtype=np.float32).reshape(B * H, S, D)
    pk = np.asarray(phases_k, dtype=np.float32).reshape(B * H, S, D)
    qr = _prep(pq)  # [16, 64, S] f64
    kr = _prep(pk)

    in_maps = []
    for c in range(N_CORES):
        p0, p1 = 2 * c, 2 * c + 1
        pin0 = np.stack([kr[p0], qr[p0]]).astype(np.float16)  # [2, 64, S]
        pin1 = np.empty((2, 2 * D, S), dtype=np.float16)  # [2, 128, S] packed
        for t, r in ((0, kr[p1]), (1, qr[p1])):
            pin1[t, :D] = (np.pi / 2) - np.abs(r)
            pin1[t, D:] = r
        in_maps.append(
            {"pin0": np.ascontiguousarray(pin0), "pin1": np.ascontiguousarray(pin1)}
        )

    if "nc" not in _NC_CACHE:
        _NC_CACHE["nc"] = build_kernel()
    nc = _NC_CACHE["nc"]

    res = run_bass_kernel_spmd(
        nc, in_maps, core_ids=list(range(N_CORES)), trace=_trace
    )
    # [16, N_BLK, 128, BLK*S] -> [16, S, S]: block j holds q-tiles
    # (BLK*j+i) in column slices i*S:(i+1)*S.
    full = np.concatenate([r["out"] for r in res.results], axis=0)
    full = full.reshape(B * H, N_BLK, Q_TILE, BLK, S)
    full = full.transpose(0, 1, 3, 2, 4).reshape(B * H, S, S)
    out = full.astype(np.float32).reshape(B, H, S, S)
    if _trace:
        return out, res
    return out


# revision 13
# speedup vs baseline: 1.1671x; 1.1671x over previous
"""Trainium2 Bass kernel for PhaseCoherenceComputer.

coherence[b,h,q,k] = mean_d cos(phases_q[b,h,q,d] - phases_k[b,h,k,d])
                   = (cos_q @ cos_k^T + sin_q @ sin_k^T) / 64

Shapes: phases_q/k [2, 8, 2048, 64] f32 -> out [2, 8, 2048, 2048] f32.

Strategy (8 NeuronCores, data-parallel over the 16 (b,h) pairs, 2 per core):
- f16 everywhere off-chip (harness tolerance is 2e-2, f16 adds ~2e-4):
  per core 16.8 MB out + 1.5 MB in vs 33.5 MB + 2 MB for the f32
  baseline. The kernel is HBM-write-bound at ~358 GB/s/core, so bytes =
  time; everything else is pipelined under the write stream.
- Pair 0 (ramp-critical): host ships range-reduced phases r in [-pi,pi]
  as f16 [64, S] half-tensor chunks; a DVE sign-bit clear builds |r| in
  partitions 0:64 (r in 64:128) and one Sin activation with
  per-partition (scale, bias) = (-1, pi/2)/(+1, 0) yields
  U = [cos^T; sin^T] (cos r = sin(pi/2 - |r|), arguments inside the
  accurate [-pi/2, pi/2] spline range).
- Pair 1: host pre-packs [pi/2-|r|; r] as full [128, S] blocks (plain
  Sin, no abs) so its input DMAs use all 16 SDMA ports. Its raw tiles
  alias pair-0's (raw pool bufs=1): the WAR dependency delays these
  DMAs until pair-0's sins finish, so they never steal wire time from
  the ramp-critical pair-0 input, and instead ride the saturated output
  stream. Pair-1 sins are spread one half-sin per pair-0 block.
- One K=128 f16 matmul per [128 q x 512 k] output block. PSUM is carved
  into four [128, 1024] half-tiles (2 banks each): per q-tile, psA
  holds k-blocks 0-1 and psB k-blocks 2-3, so VectorE (psA) and ACT
  (psB) recycle PSUM independently; the PSUM chain (matmul + one
  half-evac ~2.4 us per 2 tiles) stays under the DMA drain period.
  Evacuation applies the 1/64 scale and converts to f16.
- Output DMA: 2 q-tiles per [128, 2*S] f16 SBUF block, one 1 MB
  sync-ring (HWDGE) DMA with 8 KB contiguous per-partition descriptors
  (DRAM layout [8 blocks, 128, 2*S] per pair; host unpermutes). All
  output DMAs ride the otherwise-idle SP ring so ACT compute never
  delays an issue. The first and last blocks stream as 4 x 256 KB
  quarter-DMAs fired per half-evac, starting the HBM write stream ~2 us
  earlier and shrinking the final drain.
"""

import sys

import numpy as np

try:
    import concourse.bacc as bacc
except ImportError:  # fresh interpreter without the axon site path
    for _p in ("/opt/trn_rl_repo", "/root/.axon_site/_ro/trn_rl_repo"):
        if _p not in sys.path:
            sys.path.insert(0, _p)
    import concourse.bacc as bacc

import concourse.mybir as mybir
import concourse.tile as tile
from concourse.bass_utils import run_bass_kernel_spmd

F32 = mybir.dt.float32
F16 = mybir.dt.float16
U16 = mybir.dt.uint16
UV_DT = F16  # matmul operand dtype
OUT_DT = F16  # device-side output dtype (host upcasts to f32)

B, H, S, D = 2, 8, 2048, 64
N_CORES = 8
PAIRS_PER_CORE = (B * H) // N_CORES  # 2
Q_TILE = 128  # output rows per matmul (PSUM partitions)
K_TILE = 512  # output cols per matmul
N_QT = S // Q_TILE  # 16
BLK = 2  # q-tiles per output DMA block (1 MB f16)
N_BLK = N_QT // BLK  # 8
HC = S // 2  # half-row chunk for input DMA / sin
EC = 2 * K_TILE  # evac chunk (one PSUM half-tile)

_NC_CACHE = {}


def build_kernel():
    """Per-core SPMD program. pin0 [2, 64, S] f16: pair-0 range-reduced
    transposed phases (tensor 0 = k-phases/v, 1 = q-phases/u). pin1
    [2, 128, S] f16: pair-1 packed [pi/2-|r|; r] blocks. Output out
    [PAIRS, N_BLK, 128, BLK*S] f16: block j holds q-tiles
    BLK*j..BLK*j+BLK-1 side by side."""
    nc = bacc.Bacc("TRN2", target_bir_lowering=False, debug=False)
    pin0 = nc.dram_tensor("pin0", [2, 64, S], F16, kind="ExternalInput")
    pin1 = nc.dram_tensor("pin1", [2, 128, S], F16, kind="ExternalInput")
    out = nc.dram_tensor(
        "out", [PAIRS_PER_CORE, N_BLK, 128, BLK * S], OUT_DT, kind="ExternalOutput"
    )
    SIN = mybir.ActivationFunctionType.Sin

    with tile.TileContext(nc) as tc:
        with (
            tc.tile_pool(name="const", bufs=1) as cpool,
            tc.tile_pool(name="raw", bufs=2) as rawpool,
            tc.tile_pool(name="uv", bufs=2) as uvpool,
            tc.tile_pool(name="ot", bufs=5) as opool,
            tc.tile_pool(name="psum", bufs=2, space="PSUM") as ppool,
        ):
            # Per-partition Sin affine for pair 0: top half cos via
            # sin(pi/2 - |r|), bottom half sin via sin(r).
            bias = cpool.tile([128, 1], F32)
            scale = cpool.tile([128, 1], F32)
            nc.vector.memset(bias[0:64, :], np.pi / 2)
            nc.vector.memset(bias[64:128, :], 0.0)
            nc.vector.memset(scale[0:64, :], -1.0)
            nc.vector.memset(scale[64:128, :], 1.0)

            raws = {}
            uvs = {}
            for p in range(PAIRS_PER_CORE):
                raws[p] = (
                    rawpool.tile([128, S], F16, tag="vraw", name="vraw"),
                    rawpool.tile([128, S], F16, tag="uraw", name="uraw"),
                )
                uvs[p] = (
                    uvpool.tile([128, S], UV_DT, tag="v", name="v"),
                    uvpool.tile([128, S], UV_DT, tag="u", name="u"),
                )

            # Pair-0 inputs in half-tensor chunks across both HWDGE rings
            # so the first sins start as early as possible. The first
            # q-tile's psA matmuls need v h0 + u cols 0:128, psB needs v h1.
            nc.sync.dma_start(out=raws[0][0][64:128, 0:HC], in_=pin0[0, :, 0:HC])
            nc.scalar.dma_start(out=raws[0][1][64:128, 0:HC], in_=pin0[1, :, 0:HC])
            nc.sync.dma_start(out=raws[0][0][64:128, HC:S], in_=pin0[0, :, HC:S])
            nc.scalar.dma_start(out=raws[0][1][64:128, HC:S], in_=pin0[1, :, HC:S])
            # Pair-1 inputs: full-width packed blocks on the scalar ring,
            # queued behind pair-0's chunks. They drain during the ramp
            # window where the wire is mostly idle (pair-0's sins are
            # ACT-table-gated until ~11.4 us regardless), keeping the
            # output stream free of input traffic.
            nc.scalar.dma_start(out=raws[1][0][:], in_=pin1[0])
            nc.scalar.dma_start(out=raws[1][1][:], in_=pin1[1])

            def abs_step(t, h):
                hs = slice(h * HC, (h + 1) * HC)
                nc.vector.tensor_scalar(
                    raws[0][t][0:64, hs].bitcast(U16),
                    raws[0][t][64:128, hs].bitcast(U16),
                    0x7FFF,
                    None,
                    mybir.AluOpType.bitwise_and,
                )

            def sin_step(p, t, h):
                hs = slice(h * HC, (h + 1) * HC)
                if p == 0:
                    nc.scalar.activation(
                        uvs[0][t][:, hs], raws[0][t][:, hs], SIN,
                        bias=bias[:], scale=scale[:],
                    )
                else:  # host-packed: plain sin
                    nc.scalar.activation(uvs[1][t][:, hs], raws[1][t][:, hs], SIN)

            for t, h in ((0, 0), (1, 0), (0, 1), (1, 1)):
                abs_step(t, h)
                sin_step(0, t, h)

            def q_tile(p, q, ot, col0, dma_quarters):
                v, u = uvs[p][0], uvs[p][1]
                us = u[:, q * Q_TILE : (q + 1) * Q_TILE]
                psA = ppool.tile([128, EC], F32, tag="psA", name="psA")
                psB = ppool.tile([128, EC], F32, tag="psB", name="psB")
                for k in range(2):
                    nc.tensor.matmul(
                        psA[:, k * K_TILE : (k + 1) * K_TILE],
                        us,
                        v[:, k * K_TILE : (k + 1) * K_TILE],
                        start=True,
                        stop=True,
                    )
                for k in range(2):
                    nc.tensor.matmul(
                        psB[:, k * K_TILE : (k + 1) * K_TILE],
                        us,
                        v[:, (k + 2) * K_TILE : (k + 3) * K_TILE],
                        start=True,
                        stop=True,
                    )
                nc.vector.tensor_scalar_mul(ot[:, col0 : col0 + EC], psA[:], 1.0 / D)
                if dma_quarters is not None:
                    nc.sync.dma_start(
                        out=dma_quarters[col0 : col0 + EC], in_=ot[:, col0 : col0 + EC]
                    )
                nc.scalar.mul(ot[:, col0 + EC : col0 + 2 * EC], psB[:], 1.0 / D)
                if dma_quarters is not None:
                    nc.sync.dma_start(
                        out=dma_quarters[col0 + EC : col0 + 2 * EC],
                        in_=ot[:, col0 + EC : col0 + 2 * EC],
                    )

            # Pair-1 half-sins spread one per block through pair-0's
            # q-loop, after its (WAR-delayed) input DMAs have landed and
            # early enough to finish before pair-0's last block.
            prep1 = {8: (1, 0, 0), 10: (1, 0, 1), 12: (1, 1, 0), 14: (1, 1, 1)}

            for p in range(PAIRS_PER_CORE):
                for blk in range(N_BLK):
                    ot = opool.tile([128, BLK * S], OUT_DT, tag="ot", name="ot")
                    split = (p == 0 and blk == 0) or (
                        p == PAIRS_PER_CORE - 1 and blk == N_BLK - 1
                    )
                    dq = None
                    if split:
                        class _Q:  # column-sliced DMA target for this block
                            def __getitem__(_s, cols):
                                return out[p, blk, :, cols]
                        dq = _Q()
                    for j in range(BLK):
                        q = blk * BLK + j
                        q_tile(p, q, ot, j * S, dq)
                        if p == 0 and q in prep1:
                            sin_step(*prep1[q])
                    if not split:
                        nc.sync.dma_start(out=out[p, blk], in_=ot[:])
    nc.compile()
    return nc


def _prep(ph):
    """[16, S, D] phases -> [16, 64, S] f64 range-reduced transposed."""
    pht = ph.astype(np.float64).transpose(0, 2, 1)  # [16, D, S]
    return np.mod(pht + np.pi, 2 * np.pi) - np.pi


def kernel(phases_q, phases_k, _trace=False):
    pq = np.asarray(phases_q, dtype=np.float32).reshape(B * H, S, D)
    pk = np.asarray(phases_k, dtype=np.float32).reshape(B * H, S, D)
    qr = _prep(pq)  # [16, 64, S] f64
    kr = _prep(pk)

    in_maps = []
    for c in range(N_CORES):
        p0, p1 = 2 * c, 2 * c + 1
        pin0 = np.stack([kr[p0], qr[p0]]).astype(np.float16)  # [2, 64, S]
        pin1 = np.empty((2, 2 * D, S), dtype=np.float16)  # [2, 128, S] packed
        for t, r in ((0, kr[p1]), (1, qr[p1])):
            pin1[t, :D] = (np.pi / 2) - np.abs(r)
            pin1[t, D:] = r
        in_maps.append(
            {"pin0": np.ascontiguousarray(pin0), "pin1": np.ascontiguousarray(pin1)}
        )

    if "nc" not in _NC_CACHE:
        _NC_CACHE["nc"] = build_kernel()
    nc = _NC_CACHE["nc"]

    res = run_bass_kernel_spmd(
        nc, in_maps, core_ids=list(range(N_CORES)), trace=_trace
    )
    # [16, N_BLK, 128, BLK*S] -> [16, S, S]: block j holds q-tiles
    # (BLK*j+i) in column slices i*S:(i+1)*S.
    full = np.concatenate([r["out"] for r in res.results], axis=0)
    full = full.reshape(B * H, N_BLK, Q_TILE, BLK, S)
    full = full.transpose(0, 1, 3, 2, 4).reshape(B * H, S, S)
    out = full.astype(np.float32).reshape(B, H, S, S)
    if _trace:
        return out, res
    return out
